# revision 23
# baseline (speedup 1.0000x reference)
"""InternLM3 custom attention on 8 TRN2 NeuronCores.

Sharding: heads 4-per-core for K/V projection + attention (qk_w/v_w
column-parallel by head); AllToAll converts the attention output from
head-sharded to sequence-sharded; o-projection runs sequence-parallel
(full o_w per core) so each core emits a [256, 2048] output slice.

v3: software-pipelined emission. The attention inner loop is ACT-bound
(exp) while the projections are PE-bound, so the K/V projection matmuls
for seq chunk c+1 are dripped one unit at a time into chunk c's
attention emission stream -- the PE executes them in the bubbles where
it would otherwise wait on exp. Other changes vs v2: the softmax
denominator broadcast lands in the unused partitions 64..127 of the PV
PSUM bank (start=True only zeroes the partitions it writes), removing
the bcs copy and the false serialization through the scores pool; the
RoPE perm-matmul PSUM reuses the K-projection banks; triangle mask and
RoPE cos/add run on the otherwise-idle Pool engine; big loads are split
across the SP/ACT/DVE DMA queues; o-projection loops t-then-ob so the
stationary operand is reused.
"""

import sys

sys.path.insert(0, "/opt/trn_rl_repo")

import numpy as np
import ml_dtypes

import concourse.bass as bass
import concourse.tile as tile
from concourse import bacc, mybir
from concourse.bass import ds, ts
from concourse.bass_utils import run_bass_kernel_spmd

F32 = mybir.dt.float32
BF16 = mybir.dt.bfloat16
NCORES = 8
S = 2048          # sequence
HID = 2048        # hidden
NH = 32           # total heads
HD = 64           # head dim
HPC = NH // NCORES      # heads per core = 4
DPC = HPC * HD          # head-dims per core = 256
SSL = S // NCORES       # output seq slice per core = 256
VW = 66                 # interleaved V stride: 64 dims + 1 ones + 1 pad
ROPE_THETA = 10000.0

# packed input blob layout (bf16 elements, per core)
_BLOB_SPEC = [
    ("hidT", HID * S),
    ("qkwT", HID * DPC),
    ("vwT", HID * DPC),
    ("owT", HID * HID),
    ("xT", DPC * S),
    ("cosT", 128 * S),
    ("sinT", 128 * S),
    ("triT", 128 * 128),
    ("permT", 128 * 128),
]
BLOB_OFFS = {}
_off = 0
for _nm, _n in _BLOB_SPEC:
    BLOB_OFFS[_nm] = (_off, _n)
    _off += _n
BLOB_ELEMS = _off


def build_program(collective=True, dbg=False):
    nc = bacc.Bacc("TRN2", target_bir_lowering=False, debug=False,
                   num_devices=NCORES)

    blob = nc.dram_tensor("blob", [BLOB_ELEMS], BF16, kind="ExternalInput").ap()

    def bslice(name):
        off, n = BLOB_OFFS[name]
        return blob[ds(off, n)]

    hidT = bslice("hidT").rearrange("(n p s) -> p n s", p=128, s=S)
    qkwT = bslice("qkwT").rearrange("(n p d) -> p n d", p=128, d=DPC)
    vwT = bslice("vwT").rearrange("(n p d) -> p n d", p=128, d=DPC)
    owT = bslice("owT").rearrange("(n p d) -> p n d", p=128, d=HID)
    xT_in = bslice("xT").rearrange("(t p s) -> p t s", p=128, s=S)
    cosT = bslice("cosT").rearrange("(p s) -> p s", p=128)
    sinT = bslice("sinT").rearrange("(p s) -> p s", p=128)
    triT = bslice("triT").rearrange("(p q) -> p q", p=128)
    permT = bslice("permT").rearrange("(p q) -> p q", p=128)
    out_sl = nc.dram_tensor("out_slice", [SSL, HID], F32,
                            kind="ExternalOutput").ap()

    with tile.TileContext(nc) as tc:
        with (
            nc.allow_low_precision(reason="bf16 streaming, fp32 psum accum"),
            tc.tile_pool(name="const", bufs=1) as const,
            tc.tile_pool(name="dram", bufs=1, space="DRAM") as dram,
        ):
            # ---- persistent SBUF residents ----
            qkw_t = const.tile([128, 16, DPC], BF16)
            vw_t = const.tile([128, 16, DPC], BF16)
            xt = const.tile([128, 2, S], BF16)      # X^T, rope'd on host
            cos_t = const.tile([128, S], BF16)
            sin_t = const.tile([128, S], BF16)
            tri_t = const.tile([128, 128], BF16)
            perm_t = const.tile([128, 128], BF16)
            ow_t = const.tile([128, 16, HID], BF16)
            kt = const.tile([128, 2, S], BF16)      # K^T, rope'd in place
            v_t = const.tile([128, 16, VW * HPC], BF16)
            att_t = const.tile([128, 2, S], BF16)   # attn^T assembled
            ones_t = const.tile([1, HD], BF16)

            nc.vector.memset(ones_t[:], 1.0)
            # ones column of V (denominator accumulator)
            nc.vector.memset(
                v_t[:].rearrange("p st (h w) -> p st h w", w=VW)[:, :, :, HD:HD + 1],
                1.0)

            with (
                tc.tile_pool(name="hidp", bufs=1) as hidp,
                tc.tile_pool(name="psk", bufs=2, space="PSUM") as psk,
                tc.tile_pool(name="psv", bufs=2, space="PSUM") as psv,
                tc.tile_pool(name="pss", bufs=2, space="PSUM") as pss,
                tc.tile_pool(name="pspv", bufs=2, space="PSUM") as pspv,
                tc.tile_pool(name="sw", bufs=2) as swp,
                tc.tile_pool(name="pp", bufs=4) as ppool,
                tc.tile_pool(name="rr", bufs=2) as rrp,
            ):
                hid_t = hidp.tile([128, 16, S], BF16)

                # ---- DMA plan ----
                # SP queue: qkw/hidden interleaved in K-proj consumption
                # order, then the later hidden chunks, then o_w (needed only
                # ~150us in).
                # SP queue: the hidden chunks (K-proj streams chunk 0 per
                # hc), then o_w in the background (needed only ~150us in).
                for (a, b) in [(0, 2), (2, 5), (5, 8), (8, 12), (12, 16)]:
                    nc.sync.dma_start(out=hid_t[:, a:b, ts(0, 512)],
                                      in_=hidT[:, a:b, ts(0, 512)])
                for sq in range(1, 4):
                    nc.sync.dma_start(out=hid_t[:, :, ts(sq, 512)],
                                      in_=hidT[:, :, ts(sq, 512)])
                nc.sync.dma_start(out=ow_t[:], in_=owT)
                # ACT queue: qk weights first (K-proj consumes them with the
                # SP hidden stream in parallel), then rope consts, V
                # weights, queries, mask.
                nc.scalar.dma_start(out=qkw_t[:, 0:2, :], in_=qkwT[:, 0:2, :])
                nc.scalar.dma_start(out=qkw_t[:, 2:16, :], in_=qkwT[:, 2:16, :])
                nc.scalar.dma_start(out=perm_t[:], in_=permT)
                nc.scalar.dma_start(out=cos_t[:], in_=cosT)
                nc.scalar.dma_start(out=sin_t[:], in_=sinT)
                nc.scalar.dma_start(out=vw_t[:], in_=vwT)
                nc.scalar.dma_start(out=xt[:], in_=xT_in)
                nc.scalar.dma_start(out=tri_t[:], in_=triT)

                # ---- phase A emission units (split K / V streams) ----
                def k_units(sq):
                    """K-projection + K-RoPE for chunk sq, in PE-sized
                    units. The RoPE perm-matmul PSUM reuses the K banks;
                    cos-mul + add run on the Pool engine."""
                    sqs = ds(512 * sq, 512)
                    pk = [psk.tile([128, 512], F32, tag='pk', name='pk')
                          for _ in range(2)]
                    for hc in range(16):
                        for m in range(2):
                            nc.tensor.matmul(
                                pk[m][:],
                                (qkw_t[:, hc, ts(m, 128)]),
                                (hid_t[:, hc, sqs]),
                                start=(hc == 0), stop=(hc == 15))
                        yield
                    ks = swp.tile([128, 2, 512], BF16, tag="sw")
                    for t in range(2):
                        nc.scalar.copy(out=kt[:, t, sqs], in_=pk[t][:])
                        yield
                    for t in range(2):
                        ksp = psk.tile([128, 512], F32, tag='pk', name='ksp')
                        nc.tensor.matmul(ksp[:], (perm_t[:]),
                                         (kt[:, t, sqs]),
                                         start=True, stop=True)
                        yield
                        nc.vector.tensor_mul(out=ks[:, t, :], in0=ksp[:],
                                             in1=sin_t[:, sqs])
                        nc.vector.tensor_mul(out=kt[:, t, sqs],
                                             in0=kt[:, t, sqs],
                                             in1=cos_t[:, sqs])
                        yield
                        nc.vector.tensor_add(out=kt[:, t, sqs],
                                             in0=kt[:, t, sqs],
                                             in1=ks[:, t, :])
                        yield

                def v_units(sq, st4_lo=0):
                    """V-projection for chunk sq. One psum group per bank
                    slot (start=True zeroes the written partitions' whole
                    bank row, so concurrent groups must not share a bank)."""
                    for st4 in range(st4_lo, 4):
                        pvt = psv.tile([128, 256], F32, tag='pv', name='pv')
                        for hh in range(2):
                            for hc in range(8 * hh, 8 * hh + 8):
                                nc.tensor.matmul(
                                    pvt[:],
                                    (hid_t[:, hc, ds(512 * sq + 128 * st4, 128)]),
                                    (vw_t[:, hc, :]),
                                    start=(hc == 0), stop=(hc == 15))
                            yield
                        nc.vector.tensor_copy(
                            out=v_t[:, sq * 4 + st4, :].rearrange(
                                "p (h w) -> p h w", w=VW)[:, :, 0:HD],
                            in_=pvt[:].rearrange("p (h d) -> p h d", d=HD))
                        yield

                def drain(gen):
                    if gen is not None:
                        for _ in gen:
                            pass

                def drip(gen, n=1):
                    if gen is None:
                        return None
                    for _ in range(n):
                        try:
                            next(gen)
                        except StopIteration:
                            return None
                    return gen

                # chunk 0 K-projection + V block 0, emitted whole; V blocks
                # 1..3 of chunk 0 drip just-in-time into the first q-block.
                drain(k_units(0))
                v0 = v_units(0)
                for _ in range(3):   # st4=0: two MM units + copy
                    next(v0)

                # ---- attention q-block j == sq, software-pipelined:
                #  - the PV matmul of slot i is emitted one slot late (via a
                #    pending queue) so QK(i+1) sits ahead of it in the PE
                #    queue and runs while exp(i) streams on ACT
                #  - V-proj of chunk sq drips just-in-time into head 0's
                #    early i slots (its output is first consumed at i=4*sq)
                #  - K-proj of chunk sq+1 drips into the remaining slots
                #  - each head's broadcast/normalize tail is deferred the
                #    same way so the PE never waits on the reciprocal
                from collections import deque
                for sq in range(4):
                    q0 = 512 * sq
                    nk = 4 * (sq + 1)
                    vstream = v0 if sq == 0 else v_units(sq)
                    vrate = {0: 3, 1: 3, 2: 2, 3: 1}[sq]
                    kstream = k_units(sq + 1) if sq < 3 else None
                    pending = deque()
                    for h in range(HPC):
                        hp = HD * (h % 2)
                        htl = h // 2
                        pvp = pspv.tile([128, 512], F32, tag='pvp', name='pvp')
                        for i in range(nk):
                            r = 128 * i - q0
                            w0 = max(r, 0)
                            sp = pss.tile([128, 512], F32, tag='sp')
                            nc.tensor.matmul(
                                sp[:, ds(w0, 512 - w0)],
                                (kt[hp:hp + HD, htl, ts(i, 128)]),
                                (xt[hp:hp + HD, htl, ds(q0 + w0, 512 - w0)]),
                                start=True, stop=True)
                            pt = ppool.tile([128, 512], BF16, tag="pt")
                            nc.scalar.activation(
                                out=pt[:, ds(w0, 512 - w0)],
                                in_=sp[:, ds(w0, 512 - w0)],
                                func=mybir.ActivationFunctionType.Exp,
                                scale=0.125)
                            if r >= 0:   # diagonal: ragged triangle mask
                                nc.gpsimd.tensor_mul(
                                    out=pt[:, ds(r, 128)], in0=pt[:, ds(r, 128)],
                                    in1=tri_t[:])
                            if pending:
                                pending.popleft()()
                            if h == 0 and vstream is not None:
                                vstream = drip(vstream, vrate)
                            else:
                                kstream = drip(kstream, 1)

                            def pv(pvp=pvp, pt=pt, h=h, i=i, w0=w0, nk=nk):
                                nc.tensor.matmul(
                                    pvp[0:HD + 1, ds(w0, 512 - w0)],
                                    (v_t[:, i, ds(VW * h, HD + 1)]),
                                    (pt[:, ds(w0, 512 - w0)]),
                                    start=(i == 0), stop=(i == nk - 1))
                            pending.append(pv)
                            if i == nk - 1:
                                # denominator tail: reciprocal of row 64,
                                # broadcast back into partitions 64..127 of
                                # the same psum bank (start=True only zeroes
                                # the partitions it writes), then normalize.
                                # Split into two pending slots so the PE-side
                                # broadcast lands well after the reciprocal.
                                recbox = []

                                def tail_rec(pvp=pvp, recbox=recbox):
                                    rec = rrp.tile([1, 512], BF16, tag="rec")
                                    recbox.append(rec)
                                    nc.vector.reciprocal(
                                        out=rec[:], in_=pvp[HD:HD + 1, :])

                                def tail_bc(pvp=pvp, recbox=recbox):
                                    nc.tensor.matmul(
                                        pvp[HD:128, :], (ones_t[:]),
                                        (recbox[0][:]), start=True, stop=True)

                                def tail_mul(pvp=pvp, hp=hp, htl=htl, q0=q0):
                                    # DVE may read only one PSUM operand:
                                    # stage the broadcast through SBUF.
                                    bcs = rrp.tile([HD, 512], BF16, tag="bcs")
                                    nc.vector.tensor_copy(out=bcs[:],
                                                          in_=pvp[HD:128, :])
                                    nc.vector.tensor_mul(
                                        out=att_t[hp:hp + HD, htl, ds(q0, 512)],
                                        in0=pvp[0:HD, :],
                                        in1=bcs[:])
                                pending.append(tail_rec)
                                pending.append(tail_bc)
                                pending.append(tail_mul)
                    # flush: last PVs + tails, then the remaining projection
                    # units (chunk sq+1 K and chunk sq V must be complete
                    # before the next q-block consumes them)
                    while pending:
                        pending.popleft()()
                        kstream = drip(kstream, 1)
                    drain(vstream)
                    drain(kstream)

            # =========== AllToAll: head-sharded -> seq-sharded ===========
            a2a_in = [dram.tile([NCORES, 128, SSL], BF16, name=f"a2ain{t}")
                      for t in range(2)]
            a2a_out = [dram.tile([NCORES * 128, SSL], BF16, name=f"a2aout{t}")
                       for t in range(2)]
            for t in range(2):
                nc.sync.dma_start(
                    out=a2a_in[t][:].rearrange("d p s -> p d s"),
                    in_=att_t[:, t, :].rearrange("p (d s) -> p d s", d=NCORES))
                if collective:
                    nc.gpsimd.collective_compute(
                        "AllToAll",
                        mybir.AluOpType.bypass,
                        replica_groups=[list(range(NCORES))],
                        ins=[a2a_in[t][:].opt()],
                        outs=[a2a_out[t][:].opt()],
                    )
                else:
                    # timeline-sim mock: same-size DRAM->DRAM move
                    nc.sync.dma_start(
                        out=a2a_out[t][:],
                        in_=a2a_in[t][:].rearrange("d p s -> (d p) s"))

            # =========== o-projection (sequence-parallel) ===========
            with (
                tc.tile_pool(name="af", bufs=1) as afp,
                tc.tile_pool(name="pso", bufs=8, space="PSUM") as pso,
            ):
                afull = afp.tile([128, 16, SSL], BF16)
                # a2a_out[t] rows (d p) hold global attn dims 256d+128t+p
                # -> afull n slices t, t+2, t+4, ...
                nc.sync.dma_start(
                    out=afull[:].rearrange(
                        "p (d u) s -> p d u s", u=2)[:, :, 0, :],
                    in_=a2a_out[0][:].rearrange("(d p) s -> p d s", p=128))
                nc.scalar.dma_start(
                    out=afull[:].rearrange(
                        "p (d u) s -> p d u s", u=2)[:, :, 1, :],
                    in_=a2a_out[1][:].rearrange("(d p) s -> p d s", p=128))
                po = [[pso.tile([128, 512], F32, tag='po', name='po')
                       for t in range(2)] for ob in range(4)]
                # Even hc chunks depend only on the t=0 collective, odd on
                # t=1: run all even ones first (hc-major, stationary operand
                # reused across the 4 ob tiles) so they overlap collective 1;
                # then finish each accumulation group in turn and stream its
                # fp32 PSUM straight to DRAM.
                for hc in [2 * i for i in range(8)]:
                    for t in range(2):
                        for ob in range(4):
                            nc.tensor.matmul(
                                po[ob][t][:],
                                (afull[:, hc, ts(t, 128)]),
                                (ow_t[:, hc, ts(ob, 512)]),
                                start=(hc == 0), stop=False)
                osb = afp.tile([128, 2, HID], F32)
                q = 0
                for t in range(2):
                    for ob in range(4):
                        for hc in [2 * i + 1 for i in range(8)]:
                            nc.tensor.matmul(
                                po[ob][t][:],
                                (afull[:, hc, ts(t, 128)]),
                                (ow_t[:, hc, ts(ob, 512)]),
                                start=False, stop=(hc == 15))
                        if q % 2 == 0:
                            nc.scalar.copy(out=osb[:, t, ts(ob, 512)],
                                           in_=po[ob][t][:])
                        else:
                            nc.vector.tensor_copy(out=osb[:, t, ts(ob, 512)],
                                                  in_=po[ob][t][:])
                        eng = nc.sync if q % 2 == 0 else nc.scalar
                        eng.dma_start(out=out_sl[ts(t, 128), ts(ob, 512)],
                                      in_=osb[:, t, ts(ob, 512)])
                        q += 1

    nc.compile()
    return nc


_PROGRAM = None


def _host_inputs(hidden_states, qk_w, v_w, o_w, position_ids):
    bf16 = ml_dtypes.bfloat16
    hs = np.asarray(hidden_states, dtype=np.float32)[0]          # [S, HID]
    qk_w = np.asarray(qk_w, dtype=np.float32)
    v_w = np.asarray(v_w, dtype=np.float32)
    o_w = np.asarray(o_w, dtype=np.float32)
    pos = np.asarray(position_ids)[0].astype(np.float64)         # [S]

    hidT = np.ascontiguousarray(hs.T)                            # [HID, S]
    hidT_bf = hidT.astype(bf16)
    owT_bf = np.ascontiguousarray(o_w.T).astype(bf16)            # [HID, HID]

    inv_freq = 1.0 / (ROPE_THETA ** (np.arange(0, HD, 2, dtype=np.float64) / HD))
    freqs = pos[None, :] * inv_freq[:, None]                     # [32, S]
    emb = np.concatenate([freqs, freqs], axis=0)                 # [64, S]
    cos1 = np.cos(emb).astype(np.float32)
    sin1 = np.sin(emb).astype(np.float32)
    sin_signed = sin1.copy()
    sin_signed[:HD // 2] *= -1.0                                 # fold rotate sign
    cosT = np.tile(cos1, (2, 1)).astype(bf16)                    # [128, S]
    sinT = np.tile(sin_signed, (2, 1)).astype(bf16)

    kl = np.arange(128)[:, None]
    u = np.arange(128)[None, :]
    triT = (u >= kl).astype(bf16)                                # [128, 128]

    # rotate-half row permutation (symmetric involution, per 64-row head)
    idx = np.arange(128)
    src = (idx // HD) * HD + (idx % HD + HD // 2) % HD
    permT = np.zeros((128, 128), np.float32)
    permT[idx, src] = 1.0
    permT = permT.astype(bf16)

    in_maps = []
    for c in range(NCORES):
        rows = slice(DPC * c, DPC * (c + 1))
        xT = hidT[rows]                                          # [256, S] fp32
        # host-side X RoPE: x*cos + rotate_half(x)*sin per 64-row head
        xTs = np.empty_like(xT)
        for h in range(HPC):
            b = HD * h
            xTs[b:b + 32] = -xT[b + 32:b + HD]
            xTs[b + 32:b + HD] = xT[b:b + 32]
        cs = np.tile(cos1, (HPC, 1))                             # [256, S]
        sn = np.tile(sin1, (HPC, 1))
        xTr = (xT * cs + xTs * sn).astype(bf16)
        parts = {
            "hidT": hidT_bf,
            "qkwT": np.ascontiguousarray(qk_w[rows].T).astype(bf16),
            "vwT": np.ascontiguousarray(v_w[rows].T).astype(bf16),
            "owT": owT_bf,
            "xT": xTr,
            "cosT": cosT,
            "sinT": sinT,
            "triT": triT,
            "permT": permT,
        }
        blob = np.concatenate([parts[nm].ravel() for nm, _ in _BLOB_SPEC])
        assert blob.size == BLOB_ELEMS
        in_maps.append({"blob": blob})
    return in_maps


def kernel(hidden_states, qk_w, v_w, o_w, position_ids, **extra):
    global _PROGRAM
    if _PROGRAM is None:
        _PROGRAM = build_program()
    in_maps = _host_inputs(hidden_states, qk_w, v_w, o_w, position_ids)
    res = run_bass_kernel_spmd(_PROGRAM, in_maps, list(range(NCORES)))
    out = np.concatenate([res.results[c]["out_slice"]
                          for c in range(NCORES)], axis=0)
    return out.reshape(1, S, HID).astype(np.float32)


# revision 28
# speedup vs baseline: 1.0873x; 1.0873x over previous
"""InternLM3 custom attention on 8 TRN2 NeuronCores.

Sharding: heads 4-per-core for K/V projection + attention (qk_w/v_w
column-parallel by head); AllToAll converts the attention output from
head-sharded to sequence-sharded; o-projection runs sequence-parallel
(full o_w per core) so each core emits a [256, 2048] output slice.

v3: software-pipelined emission. The attention inner loop is ACT-bound
(exp) while the projections are PE-bound, so the K/V projection matmuls
for seq chunk c+1 are dripped one unit at a time into chunk c's
attention emission stream -- the PE executes them in the bubbles where
it would otherwise wait on exp. Other changes vs v2: the softmax
denominator broadcast lands in the unused partitions 64..127 of the PV
PSUM bank (start=True only zeroes the partitions it writes), removing
the bcs copy and the false serialization through the scores pool; the
RoPE perm-matmul PSUM reuses the K-projection banks; triangle mask and
RoPE cos/add run on the otherwise-idle Pool engine; big loads are split
across the SP/ACT/DVE DMA queues; o-projection loops t-then-ob so the
stationary operand is reused.
"""

import sys

sys.path.insert(0, "/opt/trn_rl_repo")

import numpy as np
import ml_dtypes

import concourse.bass as bass
import concourse.tile as tile
from concourse import bacc, mybir
from concourse.bass import ds, ts
from concourse.bass_utils import run_bass_kernel_spmd

F32 = mybir.dt.float32
BF16 = mybir.dt.bfloat16
NCORES = 8
S = 2048          # sequence
HID = 2048        # hidden
NH = 32           # total heads
HD = 64           # head dim
HPC = NH // NCORES      # heads per core = 4
DPC = HPC * HD          # head-dims per core = 256
SSL = S // NCORES       # output seq slice per core = 256
VW = 66                 # interleaved V stride: 64 dims + 1 ones + 1 pad
ROPE_THETA = 10000.0

# packed input blob layout (bf16 elements, per core)
_BLOB_SPEC = [
    ("hidT", HID * S),
    ("qkwT", HID * DPC),
    ("vwT", HID * DPC),
    ("owT", HID * HID),
    ("xT", DPC * S),
    ("cosT", 128 * S),
    ("sinT", 128 * S),
    ("triT", 128 * 128),
    ("permT", 128 * 128),
]
BLOB_OFFS = {}
_off = 0
for _nm, _n in _BLOB_SPEC:
    BLOB_OFFS[_nm] = (_off, _n)
    _off += _n
BLOB_ELEMS = _off


def build_program(collective=True, dbg=False):
    nc = bacc.Bacc("TRN2", target_bir_lowering=False, debug=False,
                   num_devices=NCORES)

    blob = nc.dram_tensor("blob", [BLOB_ELEMS], BF16, kind="ExternalInput").ap()

    def bslice(name):
        off, n = BLOB_OFFS[name]
        return blob[ds(off, n)]

    hidT = bslice("hidT").rearrange("(n p s) -> p n s", p=128, s=S)
    qkwT = bslice("qkwT").rearrange("(n p d) -> p n d", p=128, d=DPC)
    vwT = bslice("vwT").rearrange("(n p d) -> p n d", p=128, d=DPC)
    owT = bslice("owT").rearrange("(n p d) -> p n d", p=128, d=HID)
    xT_in = bslice("xT").rearrange("(t p s) -> p t s", p=128, s=S)
    cosT = bslice("cosT").rearrange("(p s) -> p s", p=128)
    sinT = bslice("sinT").rearrange("(p s) -> p s", p=128)
    triT = bslice("triT").rearrange("(p q) -> p q", p=128)
    permT = bslice("permT").rearrange("(p q) -> p q", p=128)
    out_sl = nc.dram_tensor("out_slice", [SSL, HID], F32,
                            kind="ExternalOutput").ap()

    with tile.TileContext(nc) as tc:
        with (
            nc.allow_low_precision(reason="bf16 streaming, fp32 psum accum"),
            tc.tile_pool(name="const", bufs=1) as const,
            tc.tile_pool(name="dram", bufs=1, space="DRAM") as dram,
        ):
            # ---- persistent SBUF residents ----
            qkw_t = const.tile([128, 16, DPC], BF16)
            vw_t = const.tile([128, 16, DPC], BF16)
            xt = const.tile([128, 2, S], BF16)      # X^T, rope'd on host
            cos_t = const.tile([128, S], BF16)
            sin_t = const.tile([128, S], BF16)
            tri_t = const.tile([128, 128], BF16)
            perm_t = const.tile([128, 128], BF16)
            ow_t = const.tile([128, 16, HID], BF16)
            kt = const.tile([128, 2, S], BF16)      # K^T, rope'd in place
            v_t = const.tile([128, 16, VW * HPC], BF16)
            att_t = const.tile([128, 2, S], BF16)   # attn^T assembled
            ones_t = const.tile([1, HD], BF16)

            nc.vector.memset(ones_t[:], 1.0)
            # ones column of V (denominator accumulator)
            nc.vector.memset(
                v_t[:].rearrange("p st (h w) -> p st h w", w=VW)[:, :, :, HD:HD + 1],
                1.0)

            with (
                tc.tile_pool(name="hidp", bufs=1) as hidp,
                tc.tile_pool(name="pss", bufs=2, space="PSUM") as pss,
                tc.tile_pool(name="pspv", bufs=2, space="PSUM") as pspv,
                tc.tile_pool(name="sw", bufs=2) as swp,
                tc.tile_pool(name="pp", bufs=4) as ppool,
                tc.tile_pool(name="rr", bufs=2) as rrp,
            ):
                hid_t = hidp.tile([128, 16, S], BF16)

                # ---- DMA plan ----
                # SP queue: qkw/hidden interleaved in K-proj consumption
                # order, then the later hidden chunks, then o_w (needed only
                # ~150us in).
                # SP queue: the hidden chunks (K-proj streams chunk 0 per
                # hc), then o_w in the background (needed only ~150us in).
                for (a, b) in [(0, 2), (2, 5), (5, 8), (8, 12), (12, 16)]:
                    nc.sync.dma_start(out=hid_t[:, a:b, ts(0, 512)],
                                      in_=hidT[:, a:b, ts(0, 512)])
                for sq in range(1, 4):
                    nc.sync.dma_start(out=hid_t[:, :, ts(sq, 512)],
                                      in_=hidT[:, :, ts(sq, 512)])
                nc.sync.dma_start(out=ow_t[:], in_=owT)
                # ACT queue: qk weights first (K-proj consumes them with the
                # SP hidden stream in parallel), then rope consts, V
                # weights, queries, mask.
                nc.scalar.dma_start(out=qkw_t[:, 0:2, :], in_=qkwT[:, 0:2, :])
                nc.scalar.dma_start(out=qkw_t[:, 2:16, :], in_=qkwT[:, 2:16, :])
                nc.scalar.dma_start(out=perm_t[:], in_=permT)
                nc.scalar.dma_start(out=cos_t[:], in_=cosT)
                nc.scalar.dma_start(out=sin_t[:], in_=sinT)
                nc.scalar.dma_start(out=vw_t[:], in_=vwT)
                nc.scalar.dma_start(out=xt[:], in_=xT_in)
                nc.scalar.dma_start(out=tri_t[:], in_=triT)

                # ---- phase A emission units (split K / V streams) ----
                pools = {}

                def k_units(sq):
                    """K-projection + K-RoPE for chunk sq, in PE-sized
                    units. The RoPE perm-matmul PSUM reuses the K banks."""
                    psk = pools["psk"]
                    sqs = ds(512 * sq, 512)
                    pk = [psk.tile([128, 512], F32, tag='pk', name='pk')
                          for _ in range(2)]
                    for hc in range(16):
                        for m in range(2):
                            nc.tensor.matmul(
                                pk[m][:],
                                (qkw_t[:, hc, ts(m, 128)]),
                                (hid_t[:, hc, sqs]),
                                start=(hc == 0), stop=(hc == 15))
                        yield
                    ks = swp.tile([128, 2, 512], BF16, tag="sw")
                    for t in range(2):
                        nc.scalar.copy(out=kt[:, t, sqs], in_=pk[t][:])
                        yield
                    for t in range(2):
                        ksp = pools["psk"].tile([128, 512], F32, tag='pk',
                                                name='ksp')
                        nc.tensor.matmul(ksp[:], (perm_t[:]),
                                         (kt[:, t, sqs]),
                                         start=True, stop=True)
                        yield
                        nc.vector.tensor_mul(out=ks[:, t, :], in0=ksp[:],
                                             in1=sin_t[:, sqs])
                        nc.vector.tensor_mul(out=kt[:, t, sqs],
                                             in0=kt[:, t, sqs],
                                             in1=cos_t[:, sqs])
                        yield
                        nc.vector.tensor_add(out=kt[:, t, sqs],
                                             in0=kt[:, t, sqs],
                                             in1=ks[:, t, :])
                        yield

                def v_units(sq, st4_lo=0):
                    """V-projection for chunk sq. One psum group per bank
                    slot (start=True zeroes the written partitions' whole
                    bank row, so concurrent groups must not share a bank)."""
                    for st4 in range(st4_lo, 4):
                        pvt = pools["psv"].tile([128, 256], F32, tag='pv',
                                                name='pv')
                        for hh in range(2):
                            for hc in range(8 * hh, 8 * hh + 8):
                                nc.tensor.matmul(
                                    pvt[:],
                                    (hid_t[:, hc, ds(512 * sq + 128 * st4, 128)]),
                                    (vw_t[:, hc, :]),
                                    start=(hc == 0), stop=(hc == 15))
                            yield
                        nc.vector.tensor_copy(
                            out=v_t[:, sq * 4 + st4, :].rearrange(
                                "p (h w) -> p h w", w=VW)[:, :, 0:HD],
                            in_=pvt[:].rearrange("p (h d) -> p h d", d=HD))
                        yield

                def drain(gen):
                    if gen is not None:
                        for _ in gen:
                            pass

                def drip(gen, n=1):
                    if gen is None:
                        return None
                    for _ in range(n):
                        try:
                            next(gen)
                        except StopIteration:
                            return None
                    return gen

                # ---- attention emission, software-pipelined:
                #  - the PV matmul of slot i is emitted one slot late (via a
                #    pending queue) so QK(i+1) sits ahead of it in the PE
                #    queue and runs while exp(i) streams on ACT
                #  - V-proj of chunk sq drips just-in-time into head 0's
                #    early i slots (its output is first consumed at i=4*sq)
                #  - K-proj of chunk sq+1 drips into the remaining slots
                #  - each head's broadcast/normalize tail is deferred the
                #    same way so the PE never waits on the reciprocal
                #  - the last q-block's heads 1..3 (ACT-bound, no drip
                #    material left) run with PAIRED exps: the scores of two
                #    adjacent k-blocks land in one [128,1024] PSUM tile
                #    spanning two banks and share a single Exp instruction
                from collections import deque
                st = {"pending": deque(), "vstream": None, "vrate": 1,
                      "kstream": None, "pspair": None}

                def slot_fill(h):
                    if st["pending"]:
                        st["pending"].popleft()()
                    if h == 0 and st["vstream"] is not None:
                        st["vstream"] = drip(st["vstream"], st["vrate"])
                    else:
                        st["kstream"] = drip(st["kstream"], 1)

                def push_pv(pvp, pt_ap, h, i, w0, nk):
                    def pv():
                        nc.tensor.matmul(
                            pvp[0:HD + 1, ds(w0, 512 - w0)],
                            (v_t[:, i, ds(VW * h, HD + 1)]),
                            pt_ap,
                            start=(i == 0), stop=(i == nk - 1))
                    st["pending"].append(pv)

                def push_tail(pvp, hp, htl, q0):
                    # denominator tail: reciprocal of row 64, broadcast back
                    # into partitions 64..127 of the same psum bank
                    # (start=True only zeroes the partitions it writes),
                    # then normalize. Three pending slots so the PE-side
                    # broadcast lands well after the reciprocal.
                    recbox = []

                    def tail_rec():
                        rec = rrp.tile([1, 512], BF16, tag="rec")
                        recbox.append(rec)
                        nc.vector.reciprocal(out=rec[:], in_=pvp[HD:HD + 1, :])

                    def tail_bc():
                        nc.tensor.matmul(pvp[HD:128, :], (ones_t[:]),
                                         (recbox[0][:]), start=True, stop=True)

                    def tail_mul():
                        # DVE may read only one PSUM operand: stage the
                        # broadcast through SBUF.
                        bcs = rrp.tile([HD, 512], BF16, tag="bcs")
                        nc.vector.tensor_copy(out=bcs[:], in_=pvp[HD:128, :])
                        nc.vector.tensor_mul(
                            out=att_t[hp:hp + HD, htl, ds(q0, 512)],
                            in0=pvp[0:HD, :],
                            in1=bcs[:])
                    st["pending"].append(tail_rec)
                    st["pending"].append(tail_bc)
                    st["pending"].append(tail_mul)

                def attn_head(sq, h):
                    q0 = 512 * sq
                    nk = 4 * (sq + 1)
                    hp = HD * (h % 2)
                    htl = h // 2
                    pvp = pspv.tile([128, 512], F32, tag='pvp', name='pvp')
                    for i in range(nk):
                        r = 128 * i - q0
                        w0 = max(r, 0)
                        sp = pss.tile([128, 512], F32, tag='sp')
                        nc.tensor.matmul(
                            sp[:, ds(w0, 512 - w0)],
                            (kt[hp:hp + HD, htl, ts(i, 128)]),
                            (xt[hp:hp + HD, htl, ds(q0 + w0, 512 - w0)]),
                            start=True, stop=True)
                        pt = ppool.tile([128, 512], BF16, tag="pt")
                        nc.scalar.activation(
                            out=pt[:, ds(w0, 512 - w0)],
                            in_=sp[:, ds(w0, 512 - w0)],
                            func=mybir.ActivationFunctionType.Exp,
                            scale=0.125)
                        if r >= 0:   # diagonal: ragged triangle mask
                            nc.gpsimd.tensor_mul(
                                out=pt[:, ds(r, 128)], in0=pt[:, ds(r, 128)],
                                in1=tri_t[:])
                        slot_fill(h)
                        push_pv(pvp, pt[:, ds(w0, 512 - w0)], h, i, w0, nk)
                        if i == nk - 1:
                            push_tail(pvp, hp, htl, q0)

                def attn_head_paired(sq, h):
                    q0 = 512 * sq
                    nk = 4 * (sq + 1)
                    hp = HD * (h % 2)
                    htl = h // 2
                    pspair = st["pspair"]
                    pvp = pspv.tile([128, 512], F32, tag='pvp', name='pvp')
                    for j in range(nk // 2):
                        i0 = 2 * j
                        sp2 = pspair.tile([128, 1024], F32, tag='sp2')
                        w0s = []
                        for u in range(2):
                            i = i0 + u
                            w0 = max(128 * i - q0, 0)
                            w0s.append(w0)
                            nc.tensor.matmul(
                                sp2[:, ds(512 * u + w0, 512 - w0)],
                                (kt[hp:hp + HD, htl, ts(i, 128)]),
                                (xt[hp:hp + HD, htl, ds(q0 + w0, 512 - w0)]),
                                start=True, stop=True)
                        wlo = w0s[0]
                        pt2 = ppool.tile([128, 1024], BF16, tag="pt2")
                        nc.scalar.activation(
                            out=pt2[:, ds(wlo, 1024 - wlo)],
                            in_=sp2[:, ds(wlo, 1024 - wlo)],
                            func=mybir.ActivationFunctionType.Exp,
                            scale=0.125)
                        for u in range(2):
                            r = 128 * (i0 + u) - q0
                            if r >= 0:
                                nc.gpsimd.tensor_mul(
                                    out=pt2[:, ds(512 * u + r, 128)],
                                    in0=pt2[:, ds(512 * u + r, 128)],
                                    in1=tri_t[:])
                        slot_fill(h)
                        if st["pending"]:
                            st["pending"].popleft()()
                        for u in range(2):
                            i = i0 + u
                            w0 = w0s[u]
                            push_pv(pvp,
                                    pt2[:, ds(512 * u + w0, 512 - w0)],
                                    h, i, w0, nk)
                        if i0 + 1 == nk - 1:
                            push_tail(pvp, hp, htl, q0)

                def flush():
                    while st["pending"]:
                        st["pending"].popleft()()
                        st["kstream"] = drip(st["kstream"], 1)
                    drain(st["vstream"])
                    drain(st["kstream"])
                    st["vstream"] = None
                    st["kstream"] = None

                with (
                    tc.tile_pool(name="psk", bufs=2, space="PSUM") as psk_,
                    tc.tile_pool(name="psv", bufs=2, space="PSUM") as psv_,
                ):
                    pools["psk"] = psk_
                    pools["psv"] = psv_
                    # chunk 0 K-projection + V block 0, emitted whole; V
                    # blocks 1..3 of chunk 0 drip into the first q-block.
                    drain(k_units(0))
                    v0 = v_units(0)
                    for _ in range(3):   # st4=0: two MM units + copy
                        next(v0)
                    for sq in range(3):
                        st["vstream"] = v0 if sq == 0 else v_units(sq)
                        st["vrate"] = {0: 3, 1: 3, 2: 2}[sq]
                        st["kstream"] = k_units(sq + 1)
                        for h in range(HPC):
                            attn_head(sq, h)
                        flush()
                    # last q-block, head 0: consumes the JIT-dripped V(3)
                    st["vstream"] = v_units(3)
                    st["vrate"] = 1
                    attn_head(3, 0)
                    drain(st["vstream"])
                    st["vstream"] = None
                # psk/psv closed: their 4 banks host the paired score
                # tiles for the remaining (ACT-bound) heads.
                with tc.tile_pool(name="pspair", bufs=2,
                                  space="PSUM") as pspair_:
                    st["pspair"] = pspair_
                    for h in range(1, HPC):
                        attn_head_paired(3, h)
                    flush()

            # =========== AllToAll: head-sharded -> seq-sharded ===========
            a2a_in = [dram.tile([NCORES, 128, SSL], BF16, name=f"a2ain{t}")
                      for t in range(2)]
            a2a_out = [dram.tile([NCORES * 128, SSL], BF16, name=f"a2aout{t}")
                       for t in range(2)]
            for t in range(2):
                nc.sync.dma_start(
                    out=a2a_in[t][:].rearrange("d p s -> p d s"),
                    in_=att_t[:, t, :].rearrange("p (d s) -> p d s", d=NCORES))
                if collective:
                    nc.gpsimd.collective_compute(
                        "AllToAll",
                        mybir.AluOpType.bypass,
                        replica_groups=[list(range(NCORES))],
                        ins=[a2a_in[t][:].opt()],
                        outs=[a2a_out[t][:].opt()],
                    )
                else:
                    # timeline-sim mock: same-size DRAM->DRAM move
                    nc.sync.dma_start(
                        out=a2a_out[t][:],
                        in_=a2a_in[t][:].rearrange("d p s -> (d p) s"))

            # =========== o-projection (sequence-parallel) ===========
            with (
                tc.tile_pool(name="af", bufs=1) as afp,
                tc.tile_pool(name="pso", bufs=8, space="PSUM") as pso,
            ):
                afull = afp.tile([128, 16, SSL], BF16)
                # a2a_out[t] rows (d p) hold global attn dims 256d+128t+p
                # -> afull n slices t, t+2, t+4, ...
                nc.sync.dma_start(
                    out=afull[:].rearrange(
                        "p (d u) s -> p d u s", u=2)[:, :, 0, :],
                    in_=a2a_out[0][:].rearrange("(d p) s -> p d s", p=128))
                nc.scalar.dma_start(
                    out=afull[:].rearrange(
                        "p (d u) s -> p d u s", u=2)[:, :, 1, :],
                    in_=a2a_out[1][:].rearrange("(d p) s -> p d s", p=128))
                po = [[pso.tile([128, 512], F32, tag='po', name='po')
                       for t in range(2)] for ob in range(4)]
                # Even hc chunks depend only on the t=0 collective, odd on
                # t=1: run all even ones first (hc-major, stationary operand
                # reused across the 4 ob tiles) so they overlap collective 1;
                # then finish each accumulation group in turn and stream its
                # fp32 PSUM straight to DRAM.
                for hc in [2 * i for i in range(8)]:
                    for t in range(2):
                        for ob in range(4):
                            nc.tensor.matmul(
                                po[ob][t][:],
                                (afull[:, hc, ts(t, 128)]),
                                (ow_t[:, hc, ts(ob, 512)]),
                                start=(hc == 0), stop=False)
                osb = afp.tile([128, 2, HID], F32)
                q = 0
                for t in range(2):
                    for ob in range(4):
                        for hc in [2 * i + 1 for i in range(8)]:
                            nc.tensor.matmul(
                                po[ob][t][:],
                                (afull[:, hc, ts(t, 128)]),
                                (ow_t[:, hc, ts(ob, 512)]),
                                start=False, stop=(hc == 15))
                        if q % 2 == 0:
                            nc.scalar.copy(out=osb[:, t, ts(ob, 512)],
                                           in_=po[ob][t][:])
                        else:
                            nc.vector.tensor_copy(out=osb[:, t, ts(ob, 512)],
                                                  in_=po[ob][t][:])
                        eng = nc.sync if q % 2 == 0 else nc.scalar
                        eng.dma_start(out=out_sl[ts(t, 128), ts(ob, 512)],
                                      in_=osb[:, t, ts(ob, 512)])
                        q += 1

    nc.compile()
    return nc


_PROGRAM = None


def _host_inputs(hidden_states, qk_w, v_w, o_w, position_ids):
    bf16 = ml_dtypes.bfloat16
    hs = np.asarray(hidden_states, dtype=np.float32)[0]          # [S, HID]
    qk_w = np.asarray(qk_w, dtype=np.float32)
    v_w = np.asarray(v_w, dtype=np.float32)
    o_w = np.asarray(o_w, dtype=np.float32)
    pos = np.asarray(position_ids)[0].astype(np.float64)         # [S]

    hidT = np.ascontiguousarray(hs.T)                            # [HID, S]
    hidT_bf = hidT.astype(bf16)
    owT_bf = np.ascontiguousarray(o_w.T).astype(bf16)            # [HID, HID]

    inv_freq = 1.0 / (ROPE_THETA ** (np.arange(0, HD, 2, dtype=np.float64) / HD))
    freqs = pos[None, :] * inv_freq[:, None]                     # [32, S]
    emb = np.concatenate([freqs, freqs], axis=0)                 # [64, S]
    cos1 = np.cos(emb).astype(np.float32)
    sin1 = np.sin(emb).astype(np.float32)
    sin_signed = sin1.copy()
    sin_signed[:HD // 2] *= -1.0                                 # fold rotate sign
    cosT = np.tile(cos1, (2, 1)).astype(bf16)                    # [128, S]
    sinT = np.tile(sin_signed, (2, 1)).astype(bf16)

    kl = np.arange(128)[:, None]
    u = np.arange(128)[None, :]
    triT = (u >= kl).astype(bf16)                                # [128, 128]

    # rotate-half row permutation (symmetric involution, per 64-row head)
    idx = np.arange(128)
    src = (idx // HD) * HD + (idx % HD + HD // 2) % HD
    permT = np.zeros((128, 128), np.float32)
    permT[idx, src] = 1.0
    permT = permT.astype(bf16)

    in_maps = []
    for c in range(NCORES):
        rows = slice(DPC * c, DPC * (c + 1))
        xT = hidT[rows]                                          # [256, S] fp32
        # host-side X RoPE: x*cos + rotate_half(x)*sin per 64-row head
        xTs = np.empty_like(xT)
        for h in range(HPC):
            b = HD * h
            xTs[b:b + 32] = -xT[b + 32:b + HD]
            xTs[b + 32:b + HD] = xT[b:b + 32]
        cs = np.tile(cos1, (HPC, 1))                             # [256, S]
        sn = np.tile(sin1, (HPC, 1))
        xTr = (xT * cs + xTs * sn).astype(bf16)
        parts = {
            "hidT": hidT_bf,
            "qkwT": np.ascontiguousarray(qk_w[rows].T).astype(bf16),
            "vwT": np.ascontiguousarray(v_w[rows].T).astype(bf16),
            "owT": owT_bf,
            "xT": xTr,
            "cosT": cosT,
            "sinT": sinT,
            "triT": triT,
            "permT": permT,
        }
        blob = np.concatenate([parts[nm].ravel() for nm, _ in _BLOB_SPEC])
        assert blob.size == BLOB_ELEMS
        in_maps.append({"blob": blob})
    return in_maps


def kernel(hidden_states, qk_w, v_w, o_w, position_ids, **extra):
    global _PROGRAM
    if _PROGRAM is None:
        _PROGRAM = build_program()
    in_maps = _host_inputs(hidden_states, qk_w, v_w, o_w, position_ids)
    res = run_bass_kernel_spmd(_PROGRAM, in_maps, list(range(NCORES)))
    out = np.concatenate([res.results[c]["out_slice"]
                          for c in range(NCORES)], axis=0)
    return out.reshape(1, S, HID).astype(np.float32)
